# revision 15
# baseline (speedup 1.0000x reference)
"""GATv2 layer (broadcast-score variant) as a Bass/Tile kernel on 8 NeuronCores.

Math: since scores[i,j] = e[j] (row-broadcast) masked by A, the masked softmax +
aggregation collapse to
    g = exp(e - ln2),  e = relu(X @ W.T) @ a_w          (the ln2 bias cancels)
    out = relu( (A @ (g*Wh)) / (A @ g) )                with Wh = X @ W.T
Each core computes a 1024-row block of the output.

v6 (vs ~64.7us v2 baseline):
- A / X / W repacked on the HOST into [128, *] partition-major layouts so every
  dma_start reads per-partition-contiguous bytes (128 big descriptors/trigger;
  v2's 512-descriptor triggers took ~1us each and serialized the Sync engine).
- Whole 8MB A.T block resident in SBUF; slices split across BOTH HWDGE rings
  (sync + scalar) with small leading slices so phase 2 starts early.
- xt is 16 per-batch tiles (one DMA each) -> exact Tile deps; the first matmul
  only waits for batch 0's 128KB.
- Phase-1 and phase-2 emission interleaved so the PE queue never drains.
- dn (the softmax denominator, a single output row) no longer burns a full
  DoubleRow stream against the 128-wide nm stationary: it runs as plain-fp8
  M=1 matmuls column-tiled onto 4 distinct 32-column PE strips (concurrent
  per the col_grp tiling rules), halving dn's wall cost and eliminating the
  1MB zero-padded Gg tensor + its 7.5us GpSimd memset.  The 4 per-strip
  partial dn rows are summed on DVE at the end.
"""

import numpy as np

import concourse.tile as tile
from concourse import bacc, mybir
from concourse.bass_utils import run_bass_kernel_spmd

N, IN_DIM, OUT_DIM = 8192, 256, 128
NCORES = 8
RPC = N // NCORES          # rows per core (1024)
P = 128                    # partitions
NJ = N // P                # 64 contraction chunks
DH = IN_DIM // P           # 2 chunks of the d-contraction
HF = RPC // 2              # 512-wide i-halves for phase-2 streams
LN2 = 0.6931471805599453

B1 = 4                     # phase-1 j-tile batch (chunks per batch)
NB = NJ // B1              # 16 batches
# at slice sizes (j-chunks); chunk spans assigned below, split across rings
A_SLICES = [2, 2, 4, 8, 8, 8, 8, 8, 8, 8]
A_ON_SCALAR = {4, 6, 8}    # these slice indices ride the scalar ring

F32 = mybir.dt.float32
BF16 = mybir.dt.bfloat16
FP8 = mybir.dt.float8e4
AFT = mybir.ActivationFunctionType

NM_LAG = 1                 # nm group batch-lag behind phase 1
DN_LAG = 1                 # dn chunk batch-lag behind phase 1


def emit_body(nc, tc, io, pools):
    at, xt, wt, awb, out = io
    big, ph1, outp = pools

    # ---- DMA program -------------------------------------------------------
    # sync ring head: xt batch 0, wt, first at slices, aw
    xb = [big.tile([P, DH, B1 * P], FP8, tag=f"xb{b}", name=f"xb{b}")
          for b in range(NB)]

    def xb_dma(eng, b):
        eng.dma_start(
            out=xb[b], in_=xt[:, b * DH * B1 * P:(b + 1) * DH * B1 * P]
            .rearrange("p (dh n) -> p dh n", dh=DH))

    xb_dma(nc.sync, 0)
    wt_sb = big.tile([P, DH, OUT_DIM], BF16, tag="wt_sb", name="wt_sb")
    nc.sync.dma_start(out=wt_sb, in_=wt.rearrange("p (dh o) -> p dh o", dh=DH))

    at_s = []
    at_chunk0 = []
    pos = 0

    def at_dma(s):
        nonlocal pos
        ach = A_SLICES[s]
        a4 = big.tile([P, ach, RPC], FP8, tag=f"at{s}", name=f"at{s}")
        eng = nc.scalar if s in A_ON_SCALAR else nc.sync
        eng.dma_start(
            out=a4, in_=at[:, pos * RPC:(pos + ach) * RPC]
            .rearrange("p (c i) -> p c i", c=ach))
        at_s.append(a4)
        at_chunk0.append(pos)
        pos += ach

    at_dma(0)
    aw_sb = big.tile([P, OUT_DIM], BF16, tag="aw_sb", name="aw_sb")
    nc.sync.dma_start(out=aw_sb, in_=awb[:, :])
    at_dma(1)
    # scalar ring: the rest of xt, then its share of at
    for b in range(1, NB):
        xb_dma(nc.scalar, b)
    for s in range(2, len(A_SLICES)):
        at_dma(s)

    def at_chunk(c, h):
        """[P, HF] slice of A.T chunk c, i-half h."""
        s = 0
        while at_chunk0[s] + A_SLICES[s] <= c:
            s += 1
        return at_s[s][:, c - at_chunk0[s], h * HF:(h + 1) * HF]

    def at_pair(cp, h):
        """[P, 2, HF] slice holding j-chunks (2cp, 2cp+1), i-half h."""
        c0 = 2 * cp
        s = 0
        while at_chunk0[s] + A_SLICES[s] <= c0:
            s += 1
        r = c0 - at_chunk0[s]
        return at_s[s][:, r:r + 2, h * HF:(h + 1) * HF]

    # ---- persistent SBUF state --------------------------------------------
    G = big.tile([P, NJ, OUT_DIM], FP8, tag="G", name="G")
    g8 = big.tile([P, NJ], FP8, tag="g8", name="g8")     # g as fp8, dn lhsT
    g64 = big.tile([P, NJ], F32, tag="g64", name="g64")
    ones_bf = big.tile([1, P], BF16, tag="ones", name="ones")
    nc.vector.memset(ones_bf, 1.0)
    nln2 = big.tile([P, 1], F32, tag="nln2", name="nln2")
    nc.vector.memset(nln2, -LN2)
    rc = big.tile([1, RPC], F32, tag="rc", name="rc")

    with tc.tile_pool(name="ps", bufs=1, space="PSUM") as ps:
        nm = [ps.tile([P, HF], F32, tag=f"nm{h}", name=f"nm{h}", bufs=1)
              for h in range(2)]
        dn = [ps.tile([P, HF], F32, tag=f"dn{h}", name=f"dn{h}", bufs=1)
              for h in range(2)]

        def ph1_batch(b):
            wh4 = ps.tile([P, B1, OUT_DIM], F32, tag="wh4", name="wh4", bufs=3)
            for k in range(B1):
                off = k * P
                for dh in range(DH):
                    nc.tensor.matmul(
                        wh4[:, k, :],
                        xb[b][:, dh, off:off + P],
                        wt_sb[:, dh, :],
                        start=(dh == 0),
                        stop=(dh == DH - 1),
                    )
            t0 = b * B1
            scr = ph1.tile([P, B1, OUT_DIM], FP8, name="scr")
            e4 = ph1.tile([P, B1], F32, name="e4")
            for k in range(B1):
                nc.vector.scalar_tensor_tensor(
                    out=scr[:, k, :], in0=wh4[:, k, :], scalar=0.0,
                    in1=aw_sb,
                    op0=mybir.AluOpType.max, op1=mybir.AluOpType.mult,
                    accum_out=e4[:, k:k + 1],
                )
            nc.scalar.activation(g64[:, t0:t0 + B1], e4, AFT.Exp, bias=nln2[:, 0:1])
            for k in range(B1):
                t = t0 + k
                # 3-of-4 G-copies on Scalar, 1-of-4 on DVE (DVE owns the e-STTs)
                if t % 4 == 3:
                    nc.vector.tensor_scalar_mul(
                        G[:, t, :], wh4[:, k, :], g64[:, t:t + 1]
                    )
                else:
                    nc.scalar.activation(
                        G[:, t, :], wh4[:, k, :], AFT.Copy,
                        scale=g64[:, t:t + 1],
                    )
            nc.gpsimd.tensor_copy(out=g8[:, t0:t0 + B1], in_=g64[:, t0:t0 + B1])

        def nm_group(cp):
            for h in range(2):
                nc.tensor.matmul(
                    nm[h][:, :],
                    G[:, 2 * cp:2 * cp + 2, :],
                    at_pair(cp, h),
                    start=(cp == 0),
                    stop=(cp == NJ // 2 - 1),
                    perf_mode=mybir.MatmulPerfMode.DoubleRow,
                )

        def dn_batch(b):
            # 4 chunks as plain-fp8 M=1 matmuls on 4 distinct 32-col strips;
            # the HW runs non-conflicting col_grp tiles concurrently.
            for h in range(2):
                for q in range(B1):
                    c = b * B1 + q
                    nc.tensor.matmul(
                        dn[h][32 * q:32 * q + 1, :],
                        g8[:, c:c + 1],
                        at_chunk(c, h),
                        start=(b == 0),
                        stop=(b == NB - 1),
                        tile_position=(0, 32 * q),
                    )

        for b in range(NB):
            ph1_batch(b)
            if b >= NM_LAG:
                nm_group(2 * (b - NM_LAG))
                nm_group(2 * (b - NM_LAG) + 1)
            if b >= DN_LAG:
                dn_batch(b - DN_LAG)
        # drain: dn first so the recip->broadcast->mul chain starts earlier
        for b in range(NB - DN_LAG, NB):
            dn_batch(b)
        for cp in range(2 * (NB - NM_LAG), NJ // 2):
            nm_group(cp)

        # ---- output: out = relu(nm) * (1/dn) broadcast over o ----
        rels, rbcs = [], []
        for h in range(2):
            # dn[h] = sum of the 4 per-strip partial rows (partitions 0/32/64/96)
            # (DVE may read only one PSUM operand per op -> stage 2 via SBUF)
            c32 = outp.tile([1, HF], F32, tag="c32", name="c32")
            c96 = outp.tile([1, HF], F32, tag="c96", name="c96")
            nc.scalar.activation(c32, dn[h][32:33, :], AFT.Copy)
            nc.scalar.activation(c96, dn[h][96:97, :], AFT.Copy)
            s01 = outp.tile([1, HF], F32, tag="s01", name="s01")
            s23 = outp.tile([1, HF], F32, tag="s23", name="s23")
            nc.vector.tensor_add(s01, dn[h][0:1, :], c32)
            nc.vector.tensor_add(s23, dn[h][64:65, :], c96)
            dsum = outp.tile([1, HF], F32, tag="dsum", name="dsum")
            nc.vector.tensor_add(dsum, s01, s23)
            nc.vector.reciprocal_approx_fast(
                out=rc[0:1, h * HF:(h + 1) * HF], in_=dsum
            )
            rel = outp.tile([P, HF], F32, tag="rel", name="rel")
            nc.scalar.activation(rel, nm[h], AFT.Relu)
            rcb = outp.tile([1, HF], BF16, tag="rcb", name="rcb")
            nc.vector.tensor_copy(out=rcb, in_=rc[0:1, h * HF:(h + 1) * HF])
            rbc = ps.tile([P, HF], F32, tag="rbc", name="rbc", bufs=1)
            nc.tensor.matmul(
                rbc, ones_bf[0:1, 0:P], rcb[0:1, :], start=True, stop=True,
            )
            rels.append(rel)
            rbcs.append(rbc)
        for h in range(2):
            o_sb = outp.tile([P, HF], BF16, tag="osb", name="osb")
            nc.vector.tensor_mul(o_sb, rels[h], rbcs[h])
            eng = nc.sync if h == 0 else nc.scalar
            eng.dma_start(out=out[:, h * HF:(h + 1) * HF], in_=o_sb)


def build_nc(repeat=1):
    nc = bacc.Bacc("TRN2", target_bir_lowering=False)
    # at[p, c*RPC + i] = A[core*RPC + i, c*128 + p]  (partition-major repack)
    at = nc.dram_tensor("at", [P, NJ * RPC], FP8, kind="ExternalInput")
    # xt[p, b*1024 + dh*512 + n'] = X[b*512 + n', dh*128 + p]  (batch-major)
    xt = nc.dram_tensor("xt", [P, DH * N], FP8, kind="ExternalInput")
    # wt[p, dh*128 + o] = W.T[dh*128 + p, o]
    wt = nc.dram_tensor("wt", [P, DH * OUT_DIM], BF16, kind="ExternalInput")
    awb = nc.dram_tensor("awb", [P, OUT_DIM], BF16, kind="ExternalInput")
    out = nc.dram_tensor("out", [OUT_DIM, RPC], BF16, kind="ExternalOutput")  # transposed

    with tile.TileContext(nc) as tc:
        with (
            tc.tile_pool(name="big", bufs=1) as big,
            tc.tile_pool(name="ph1", bufs=4) as ph1,
            tc.tile_pool(name="outp", bufs=2) as outp,
        ):
            for _ in range(repeat):
                emit_body(nc, tc, (at, xt, wt, awb, out), (big, ph1, outp))
    nc.compile()
    return nc


_NC_CACHE = None


def _get_nc():
    global _NC_CACHE
    if _NC_CACHE is None:
        _NC_CACHE = build_nc()
    return _NC_CACHE


def make_in_maps(X, A, W, a_w):
    X = np.ascontiguousarray(np.asarray(X, dtype=np.float32))
    A = np.ascontiguousarray(np.asarray(A, dtype=np.float32))
    W = np.ascontiguousarray(np.asarray(W, dtype=np.float32))
    a_w = np.ascontiguousarray(np.asarray(a_w, dtype=np.float32))

    bf = mybir.dt.np(BF16)
    f8 = mybir.dt.np(FP8)
    xtp = (X.T.astype(f8)                        # [256, 8192]
           .reshape(DH, P, NB, B1 * P)           # [dh, p, b, n']
           .transpose(1, 2, 0, 3)                # [p, b, dh, n']
           .reshape(P, DH * N))
    xtp = np.ascontiguousarray(xtp)
    wtp = np.ascontiguousarray(
        W.T.astype(bf)                           # [256, 128]
        .reshape(DH, P, OUT_DIM)                 # [dh, p, o]
        .transpose(1, 0, 2)                      # [p, dh, o]
        .reshape(P, DH * OUT_DIM))
    awp = np.ascontiguousarray(
        np.broadcast_to(a_w[None, :], (P, OUT_DIM)).astype(bf))

    A8 = A.astype(f8)
    in_maps = []
    for c in range(NCORES):
        blk = A8[c * RPC:(c + 1) * RPC, :]       # [i=1024, j=8192]
        atp = (blk.reshape(RPC, NJ, P)           # [i, c, p]
               .transpose(2, 1, 0)               # [p, c, i]
               .reshape(P, NJ * RPC))
        in_maps.append({
            "at": np.ascontiguousarray(atp),
            "xt": xtp, "wt": wtp, "awb": awp,
        })
    return in_maps


def kernel_with_results(X, A, W, a_w, trace=False):
    in_maps = make_in_maps(X, A, W, a_w)
    res = run_bass_kernel_spmd(_get_nc(), in_maps, list(range(NCORES)), trace=trace)
    out = np.concatenate(
        [np.ascontiguousarray(r["out"].T) for r in res.results], axis=0
    )
    return out.astype(np.float32), res


def kernel(X, A, W, a_w):
    out, _ = kernel_with_results(X, A, W, a_w)
    return out


# revision 16
# speedup vs baseline: 1.0527x; 1.0527x over previous
"""GATv2 layer (broadcast-score variant) as a Bass/Tile kernel on 8 NeuronCores.

Math: since scores[i,j] = e[j] (row-broadcast) masked by A, the masked softmax +
aggregation collapse to
    g = exp(e - ln2),  e = relu(X @ W.T) @ a_w          (the ln2 bias cancels)
    out = relu( (A @ (g*Wh)) / (A @ g) )                with Wh = X @ W.T
Each core computes a 1024-row block of the output.

v7 (vs ~64.7us v2 baseline):
- A / X / W repacked on the HOST into [128, *] partition-major layouts so every
  dma_start reads per-partition-contiguous bytes (128 big descriptors/trigger;
  v2's 512-descriptor triggers took ~1us each and serialized the Sync engine,
  starving the DMA queue).
- Whole 8MB A.T block resident in SBUF as 16 x 0.5MB slices, alternating
  between the two HWDGE rings (sync/scalar) so supply stays ~2x ahead of the
  PE's 232 GB/s steady consumption.
- xt is 16 per-batch tiles (one DMA each) -> exact Tile deps; the first matmul
  waits only for wt + batch 0's 128KB.
- Phase-1 and phase-2 emission interleaved (iter b = ph1 batch b, nm groups of
  batch b-2, dn groups of batch b-3) so the PE never waits on the DVE/ACT
  e->g->G chain and at/G dependencies have slack.
- Both nm and dn run fp8 DoubleRow (216ns per 256-contraction x 512-col pass,
  measured at peak).  A col-tiled plain-fp8 dn variant was measured: concurrent
  but MAC-rate-bound -> no faster than DR.  The 127 zero columns of Gg are the
  price of DR's full-width stationary requirement.
- Warm-up burst of dummy matmuls right after the preamble so the PE HAM clock
  gate (cold 1.2GHz -> warm 2.4GHz after ~3.4us busy) flips before real work.
"""

import numpy as np

import concourse.tile as tile
from concourse import bacc, mybir
from concourse.bass_utils import run_bass_kernel_spmd

N, IN_DIM, OUT_DIM = 8192, 256, 128
NCORES = 8
RPC = N // NCORES          # rows per core (1024)
P = 128                    # partitions
NJ = N // P                # 64 contraction chunks
DH = IN_DIM // P           # 2 chunks of the d-contraction
HF = RPC // 2              # 512-wide i-halves for phase-2 streams
LN2 = 0.6931471805599453

B1 = 4                     # phase-1 j-tile batch (chunks per batch)
NB = NJ // B1              # 16 batches
ACH = 4                    # j-chunks per at slice (0.5MB)
NSL_A = NJ // ACH          # 16 slices, even->sync ring, odd->scalar ring

F32 = mybir.dt.float32
BF16 = mybir.dt.bfloat16
FP8 = mybir.dt.float8e4
AFT = mybir.ActivationFunctionType

NM_LAG = 2                 # nm group batch-lag behind phase 1
DN_LAG = 3                 # dn group batch-lag (Gg memset + g-cast slack)
NWARM = 18                 # dummy warm-up matmuls (HAM clock-gate ramp)


def emit_body(nc, tc, io, pools):
    at, xt, wt, awb, out = io
    big, ph1, outp = pools

    # Gg memset first on the (slow-to-launch) GpSimd queue; split so the
    # leading chunks are zeroed well before the first dn LDWEIGHTS.
    Gg = big.tile([P, NJ, OUT_DIM], FP8, tag="Gg", name="Gg")
    nc.gpsimd.memset(Gg[:, 0:8, :], 0.0)
    nc.gpsimd.memset(Gg[:, 8:24, :], 0.0)
    nc.gpsimd.memset(Gg[:, 24:NJ, :], 0.0)

    ones_bf = big.tile([1, P], BF16, tag="ones", name="ones")
    nc.vector.memset(ones_bf, 1.0)

    # ---- DMA program -------------------------------------------------------
    xb = [big.tile([P, DH, B1 * P], FP8, tag=f"xb{b}", name=f"xb{b}")
          for b in range(NB)]

    def xb_dma(eng, b):
        eng.dma_start(
            out=xb[b], in_=xt[:, b * DH * B1 * P:(b + 1) * DH * B1 * P]
            .rearrange("p (dh n) -> p dh n", dh=DH))

    at_s = []

    def at_dma(s):
        a4 = big.tile([P, ACH, RPC], FP8, tag=f"at{s}", name=f"at{s}")
        eng = nc.sync if s % 2 == 0 else nc.scalar
        eng.dma_start(
            out=a4, in_=at[:, s * ACH * RPC:(s + 1) * ACH * RPC]
            .rearrange("p (c i) -> p c i", c=ACH))
        at_s.append(a4)

    # sync ring: wt, xb0, xb1, then the even at slices
    wt_sb = big.tile([P, DH, OUT_DIM], BF16, tag="wt_sb", name="wt_sb")
    nc.sync.dma_start(out=wt_sb, in_=wt.rearrange("p (dh o) -> p dh o", dh=DH))
    xb_dma(nc.sync, 0)
    xb_dma(nc.sync, 1)
    at_dma(0)
    aw_sb = big.tile([P, OUT_DIM], BF16, tag="aw_sb", name="aw_sb")
    nc.sync.dma_start(out=aw_sb, in_=awb[:, :])
    # scalar ring interleaves its (odd) at slices among the remaining xb tiles;
    # even at slices queue back-to-back on sync.
    at_dma(1)
    at_dma(2)
    xb_dma(nc.scalar, 2)
    xb_dma(nc.scalar, 3)
    at_dma(3)
    at_dma(4)
    xb_dma(nc.scalar, 4)
    xb_dma(nc.scalar, 5)
    at_dma(5)
    at_dma(6)
    for b in range(6, 10):
        xb_dma(nc.scalar, b)
    at_dma(7)
    at_dma(8)
    for b in range(10, 13):
        xb_dma(nc.scalar, b)
    at_dma(9)
    at_dma(10)
    for b in range(13, NB):
        xb_dma(nc.scalar, b)
    for s in range(11, NSL_A):
        at_dma(s)

    def at_pair(cp, h):
        """[P, 2, HF] slice holding j-chunks (2cp, 2cp+1), i-half h."""
        c0 = 2 * cp
        s, r = c0 // ACH, c0 % ACH
        return at_s[s][:, r:r + 2, h * HF:(h + 1) * HF]

    # ---- persistent SBUF state --------------------------------------------
    G = big.tile([P, NJ, OUT_DIM], FP8, tag="G", name="G")
    g64 = big.tile([P, NJ], F32, tag="g64", name="g64")
    nln2 = big.tile([P, 1], F32, tag="nln2", name="nln2")
    nc.vector.memset(nln2, -LN2)
    rc = big.tile([1, RPC], F32, tag="rc", name="rc")

    with tc.tile_pool(name="ps", bufs=1, space="PSUM") as ps:
        nm = [ps.tile([P, HF], F32, tag=f"nm{h}", name=f"nm{h}", bufs=1)
              for h in range(2)]
        dn = [ps.tile([P, HF], F32, tag=f"dn{h}", name=f"dn{h}", bufs=1)
              for h in range(2)]

        # HAM warm-up: keep the PE busy from preamble-end until real data
        # lands, so the 4096-cycle activity window flips to 2.4GHz early.
        # Dummy 1-contraction matmuls into the (otherwise end-of-kernel) rbc
        # bank; PE executes in order so no extra sync is needed.
        warm = ps.tile([P, HF], F32, tag="rbc", name="warm", bufs=1)
        for _ in range(NWARM):
            nc.tensor.matmul(
                warm[:, 0:P], ones_bf[0:1, 0:P], ones_bf[0:1, :],
                start=True, stop=True,
            )

        def ph1_batch(b):
            wh4 = ps.tile([P, B1, OUT_DIM], F32, tag="wh4", name="wh4", bufs=3)
            for k in range(B1):
                off = k * P
                for dh in range(DH):
                    nc.tensor.matmul(
                        wh4[:, k, :],
                        xb[b][:, dh, off:off + P],
                        wt_sb[:, dh, :],
                        start=(dh == 0),
                        stop=(dh == DH - 1),
                    )
            t0 = b * B1
            scr = ph1.tile([P, B1, OUT_DIM], FP8, name="scr")
            e4 = ph1.tile([P, B1], F32, name="e4")
            for k in range(B1):
                nc.vector.scalar_tensor_tensor(
                    out=scr[:, k, :], in0=wh4[:, k, :], scalar=0.0,
                    in1=aw_sb,
                    op0=mybir.AluOpType.max, op1=mybir.AluOpType.mult,
                    accum_out=e4[:, k:k + 1],
                )
            nc.scalar.activation(g64[:, t0:t0 + B1], e4, AFT.Exp, bias=nln2[:, 0:1])
            for k in range(B1):
                t = t0 + k
                # 3-of-4 G-copies on Scalar, 1-of-4 on DVE (DVE owns the e-STTs)
                if t % 4 == 3:
                    nc.vector.tensor_scalar_mul(
                        G[:, t, :], wh4[:, k, :], g64[:, t:t + 1]
                    )
                else:
                    nc.scalar.activation(
                        G[:, t, :], wh4[:, k, :], AFT.Copy,
                        scale=g64[:, t:t + 1],
                    )
            nc.gpsimd.tensor_copy(
                out=Gg[:, t0:t0 + B1, 0:1], in_=g64[:, t0:t0 + B1]
            )

        def nm_group(cp):
            for h in range(2):
                nc.tensor.matmul(
                    nm[h][:, :],
                    G[:, 2 * cp:2 * cp + 2, :],
                    at_pair(cp, h),
                    start=(cp == 0),
                    stop=(cp == NJ // 2 - 1),
                    perf_mode=mybir.MatmulPerfMode.DoubleRow,
                )

        def dn_group(cp):
            for h in range(2):
                nc.tensor.matmul(
                    dn[h][:, :],
                    Gg[:, 2 * cp:2 * cp + 2, :],
                    at_pair(cp, h),
                    start=(cp == 0),
                    stop=(cp == NJ // 2 - 1),
                    perf_mode=mybir.MatmulPerfMode.DoubleRow,
                )

        for b in range(NB):
            ph1_batch(b)
            if b >= NM_LAG:
                nm_group(2 * (b - NM_LAG))
                nm_group(2 * (b - NM_LAG) + 1)
            if b >= DN_LAG:
                dn_group(2 * (b - DN_LAG))
                dn_group(2 * (b - DN_LAG) + 1)
        # drain: dn first so the recip->broadcast->mul chain starts earlier
        for cp in range(2 * (NB - DN_LAG), NJ // 2):
            dn_group(cp)
        for cp in range(2 * (NB - NM_LAG), NJ // 2):
            nm_group(cp)

        # ---- output: out = relu(nm) * (1/dn) broadcast over o ----
        rels, rbcs = [], []
        for h in range(2):
            nc.vector.reciprocal_approx_fast(
                out=rc[0:1, h * HF:(h + 1) * HF], in_=dn[h][0:1, :]
            )
            rel = outp.tile([P, HF], F32, tag="rel", name="rel")
            nc.scalar.activation(rel, nm[h], AFT.Relu)
            rcb = outp.tile([1, HF], BF16, tag="rcb", name="rcb")
            nc.vector.tensor_copy(out=rcb, in_=rc[0:1, h * HF:(h + 1) * HF])
            rbc = ps.tile([P, HF], F32, tag="rbc", name="rbc", bufs=1)
            nc.tensor.matmul(
                rbc, ones_bf[0:1, 0:P], rcb[0:1, :], start=True, stop=True,
            )
            rels.append(rel)
            rbcs.append(rbc)
        for h in range(2):
            o_sb = outp.tile([P, HF], BF16, tag="osb", name="osb")
            nc.vector.tensor_mul(o_sb, rels[h], rbcs[h])
            eng = nc.sync if h == 0 else nc.scalar
            eng.dma_start(out=out[:, h * HF:(h + 1) * HF], in_=o_sb)


def build_nc(repeat=1):
    nc = bacc.Bacc("TRN2", target_bir_lowering=False)
    # at[p, c*RPC + i] = A[core*RPC + i, c*128 + p]  (partition-major repack)
    at = nc.dram_tensor("at", [P, NJ * RPC], FP8, kind="ExternalInput")
    # xt[p, b*1024 + dh*512 + n'] = X[b*512 + n', dh*128 + p]  (batch-major)
    xt = nc.dram_tensor("xt", [P, DH * N], FP8, kind="ExternalInput")
    # wt[p, dh*128 + o] = W.T[dh*128 + p, o]
    wt = nc.dram_tensor("wt", [P, DH * OUT_DIM], BF16, kind="ExternalInput")
    awb = nc.dram_tensor("awb", [P, OUT_DIM], BF16, kind="ExternalInput")
    out = nc.dram_tensor("out", [OUT_DIM, RPC], BF16, kind="ExternalOutput")  # transposed

    with tile.TileContext(nc) as tc:
        with (
            tc.tile_pool(name="big", bufs=1) as big,
            tc.tile_pool(name="ph1", bufs=4) as ph1,
            tc.tile_pool(name="outp", bufs=2) as outp,
        ):
            for _ in range(repeat):
                emit_body(nc, tc, (at, xt, wt, awb, out), (big, ph1, outp))
    nc.compile()
    return nc


_NC_CACHE = None


def _get_nc():
    global _NC_CACHE
    if _NC_CACHE is None:
        _NC_CACHE = build_nc()
    return _NC_CACHE


def make_in_maps(X, A, W, a_w):
    X = np.ascontiguousarray(np.asarray(X, dtype=np.float32))
    A = np.ascontiguousarray(np.asarray(A, dtype=np.float32))
    W = np.ascontiguousarray(np.asarray(W, dtype=np.float32))
    a_w = np.ascontiguousarray(np.asarray(a_w, dtype=np.float32))

    bf = mybir.dt.np(BF16)
    f8 = mybir.dt.np(FP8)
    xtp = (X.T.astype(f8)                        # [256, 8192]
           .reshape(DH, P, NB, B1 * P)           # [dh, p, b, n']
           .transpose(1, 2, 0, 3)                # [p, b, dh, n']
           .reshape(P, DH * N))
    xtp = np.ascontiguousarray(xtp)
    wtp = np.ascontiguousarray(
        W.T.astype(bf)                           # [256, 128]
        .reshape(DH, P, OUT_DIM)                 # [dh, p, o]
        .transpose(1, 0, 2)                      # [p, dh, o]
        .reshape(P, DH * OUT_DIM))
    awp = np.ascontiguousarray(
        np.broadcast_to(a_w[None, :], (P, OUT_DIM)).astype(bf))

    A8 = A.astype(f8)
    in_maps = []
    for c in range(NCORES):
        blk = A8[c * RPC:(c + 1) * RPC, :]       # [i=1024, j=8192]
        atp = (blk.reshape(RPC, NJ, P)           # [i, c, p]
               .transpose(2, 1, 0)               # [p, c, i]
               .reshape(P, NJ * RPC))
        in_maps.append({
            "at": np.ascontiguousarray(atp),
            "xt": xtp, "wt": wtp, "awb": awp,
        })
    return in_maps


def kernel_with_results(X, A, W, a_w, trace=False):
    in_maps = make_in_maps(X, A, W, a_w)
    res = run_bass_kernel_spmd(_get_nc(), in_maps, list(range(NCORES)), trace=trace)
    out = np.concatenate(
        [np.ascontiguousarray(r["out"].T) for r in res.results], axis=0
    )
    return out.astype(np.float32), res


def kernel(X, A, W, a_w):
    out, _ = kernel_with_results(X, A, W, a_w)
    return out


# revision 20
# speedup vs baseline: 1.2795x; 1.2155x over previous
"""GATv2 layer (broadcast-score variant) as a Bass/Tile kernel on 8 NeuronCores.

Math: since scores[i,j] = e[j] (row-broadcast) masked by A, the masked softmax +
aggregation collapse to
    g = exp(e - ln2),  e = relu(X @ W.T) @ a_w          (the ln2 bias cancels)
    out = relu( (A @ (g*Wh)) / (A @ g) )                with Wh = X @ W.T
Each core computes a 1024-row block of the output.

v7 (vs ~64.7us v2 baseline):
- A / X / W repacked on the HOST into [128, *] partition-major layouts so every
  dma_start reads per-partition-contiguous bytes (128 big descriptors/trigger;
  v2's 512-descriptor triggers took ~1us each and serialized the Sync engine,
  starving the DMA queue).
- Whole 8MB A.T block resident in SBUF as 16 x 0.5MB slices, alternating
  between the two HWDGE rings (sync/scalar) so supply stays ~2x ahead of the
  PE's 232 GB/s steady consumption.
- xt is 16 per-batch tiles (one DMA each) -> exact Tile deps; the first matmul
  waits only for wt + batch 0's 128KB.
- Phase-1 and phase-2 emission interleaved (iter b = ph1 batch b, nm groups of
  batch b-2, dn groups of batch b-3) so the PE never waits on the DVE/ACT
  e->g->G chain and at/G dependencies have slack.
- Both nm and dn run fp8 DoubleRow (216ns per 256-contraction x 512-col pass,
  measured at peak).  A col-tiled plain-fp8 dn variant was measured: concurrent
  but MAC-rate-bound -> no faster than DR.  The 127 zero columns of Gg are the
  price of DR's full-width stationary requirement.
- Warm-up burst of dummy matmuls right after the preamble so the PE HAM clock
  gate (cold 1.2GHz -> warm 2.4GHz after ~3.4us busy) flips before real work.
"""

import numpy as np

import concourse.tile as tile
from concourse import bacc, mybir
from concourse.bass_utils import run_bass_kernel_spmd

N, IN_DIM, OUT_DIM = 8192, 256, 128
NCORES = 8
RPC = N // NCORES          # rows per core (1024)
P = 128                    # partitions
NJ = N // P                # 64 contraction chunks
DH = IN_DIM // P           # 2 chunks of the d-contraction
HF = RPC // 2              # 512-wide i-halves for phase-2 streams
LN2 = 0.6931471805599453

B1 = 4                     # phase-1 j-tile batch (chunks per batch)
NB = NJ // B1              # 16 batches
ACH = 4                    # j-chunks per at slice (0.5MB)
NSL_A = NJ // ACH          # 16 slices, even->sync ring, odd->scalar ring

F32 = mybir.dt.float32
BF16 = mybir.dt.bfloat16
FP8 = mybir.dt.float8e4
AFT = mybir.ActivationFunctionType

NM_LAG = 2                 # nm group batch-lag behind phase 1
DN_LAG = 3                 # dn group batch-lag (Gg memset + g-cast slack)
NWARM = 36                 # dummy warm-up matmuls (HAM clock-gate ramp)
# at slices issued from the Scalar engine mid-loop (s -> after ph1 batch b).
# Nearly all triggers live on the otherwise-idle Sync engine: DMA_DIRECT2D
# occupies the issuing engine 0.6-1.3us, and Scalar's queue must stay clear
# for the phase-1 Exp/G-copy chain (v7 put 17 triggers there -> 21us PE stall).
A_ON_SCALAR = {12: 6, 13: 7, 14: 8, 15: 9}


def emit_body(nc, tc, io, pools):
    at, xt, wt, awb, out = io
    big, ph1, outp = pools

    # Gg memset first on the (slow-to-launch) GpSimd queue; split so the
    # leading chunks are zeroed well before the first dn LDWEIGHTS.
    Gg = big.tile([P, NJ, OUT_DIM], FP8, tag="Gg", name="Gg")
    nc.gpsimd.memset(Gg[:, 0:4, :], 0.0)
    nc.gpsimd.memset(Gg[:, 4:16, :], 0.0)
    nc.gpsimd.memset(Gg[:, 16:NJ, :], 0.0)

    ones_bf = big.tile([1, P], BF16, tag="ones", name="ones")
    nc.vector.memset(ones_bf, 1.0)

    # ---- DMA program -------------------------------------------------------
    xb = [big.tile([P, DH, B1 * P], FP8, tag=f"xb{b}", name=f"xb{b}")
          for b in range(NB)]

    def xb_dma(eng, b):
        eng.dma_start(
            out=xb[b], in_=xt[:, b * DH * B1 * P:(b + 1) * DH * B1 * P]
            .rearrange("p (dh n) -> p dh n", dh=DH))

    at_s = {}

    def at_dma(eng, s):
        a4 = big.tile([P, ACH, RPC], FP8, tag=f"at{s}", name=f"at{s}")
        eng.dma_start(
            out=a4, in_=at[:, s * ACH * RPC:(s + 1) * ACH * RPC]
            .rearrange("p (c i) -> p c i", c=ACH))
        at_s[s] = a4

    # sync ring, interleaving xb tiles (tiny, early deadlines) with at slices
    # (consumed ~0.5MB / 2.2us in order); slices 12-15 come from Scalar later.
    wt_sb = big.tile([P, DH, OUT_DIM], BF16, tag="wt_sb", name="wt_sb")
    nc.sync.dma_start(out=wt_sb, in_=wt.rearrange("p (dh o) -> p dh o", dh=DH))
    xb_dma(nc.sync, 0)
    xb_dma(nc.sync, 1)
    xb_dma(nc.sync, 2)
    at_dma(nc.sync, 0)
    aw_sb = big.tile([P, OUT_DIM], BF16, tag="aw_sb", name="aw_sb")
    nc.sync.dma_start(out=aw_sb, in_=awb[:, :])
    at_dma(nc.sync, 1)
    sync_tail = [
        ("x", 3), ("x", 4), ("a", 2), ("a", 3), ("x", 5), ("x", 6),
        ("a", 4), ("a", 5), ("x", 7), ("x", 8), ("a", 6), ("a", 7),
        ("x", 9), ("x", 10), ("a", 8), ("a", 9), ("x", 11), ("x", 12),
        ("a", 10), ("a", 11), ("x", 13), ("x", 14), ("x", 15),
    ]
    for kind, idx in sync_tail:
        if kind == "x":
            xb_dma(nc.sync, idx)
        else:
            at_dma(nc.sync, idx)

    def at_pair(cp, h):
        """[P, 2, HF] slice holding j-chunks (2cp, 2cp+1), i-half h."""
        c0 = 2 * cp
        s, r = c0 // ACH, c0 % ACH
        return at_s[s][:, r:r + 2, h * HF:(h + 1) * HF]

    # ---- persistent SBUF state --------------------------------------------
    G = big.tile([P, NJ, OUT_DIM], FP8, tag="G", name="G")
    g64 = big.tile([P, NJ], F32, tag="g64", name="g64")
    nln2 = big.tile([P, 1], F32, tag="nln2", name="nln2")
    nc.vector.memset(nln2, -LN2)
    rc = big.tile([1, RPC], F32, tag="rc", name="rc")

    with tc.tile_pool(name="ps", bufs=1, space="PSUM") as ps:
        nm = [ps.tile([P, HF], F32, tag=f"nm{h}", name=f"nm{h}", bufs=1)
              for h in range(2)]
        dn = [ps.tile([P, HF], F32, tag=f"dn{h}", name=f"dn{h}", bufs=1)
              for h in range(2)]

        # HAM warm-up: keep the PE busy from preamble-end until real data
        # lands, so the 4096-cycle activity window flips to 2.4GHz early.
        # Dummy 1-contraction matmuls into the (otherwise end-of-kernel) rbc
        # bank; PE executes in order so no extra sync is needed.
        warm = ps.tile([P, HF], F32, tag="rbc", name="warm", bufs=1)
        for _ in range(NWARM):
            nc.tensor.matmul(
                warm[:, 0:P], ones_bf[0:1, 0:P], ones_bf[0:1, :],
                start=True, stop=True,
            )

        def ph1_batch(b):
            wh4 = ps.tile([P, B1, OUT_DIM], F32, tag="wh4", name="wh4", bufs=3)
            for k in range(B1):
                off = k * P
                for dh in range(DH):
                    nc.tensor.matmul(
                        wh4[:, k, :],
                        xb[b][:, dh, off:off + P],
                        wt_sb[:, dh, :],
                        start=(dh == 0),
                        stop=(dh == DH - 1),
                    )
            t0 = b * B1
            scr = ph1.tile([P, B1, OUT_DIM], FP8, name="scr")
            e4 = ph1.tile([P, B1], F32, name="e4")
            for k in range(B1):
                nc.vector.scalar_tensor_tensor(
                    out=scr[:, k, :], in0=wh4[:, k, :], scalar=0.0,
                    in1=aw_sb,
                    op0=mybir.AluOpType.max, op1=mybir.AluOpType.mult,
                    accum_out=e4[:, k:k + 1],
                )
            nc.scalar.activation(g64[:, t0:t0 + B1], e4, AFT.Exp, bias=nln2[:, 0:1])
            for k in range(B1):
                t = t0 + k
                # 3-of-4 G-copies on Scalar, 1-of-4 on DVE (DVE owns the e-STTs)
                if t % 4 == 3:
                    nc.vector.tensor_scalar_mul(
                        G[:, t, :], wh4[:, k, :], g64[:, t:t + 1]
                    )
                else:
                    nc.scalar.activation(
                        G[:, t, :], wh4[:, k, :], AFT.Copy,
                        scale=g64[:, t:t + 1],
                    )
            nc.gpsimd.tensor_copy(
                out=Gg[:, t0:t0 + B1, 0:1], in_=g64[:, t0:t0 + B1]
            )

        def nm_group(cp):
            for h in range(2):
                nc.tensor.matmul(
                    nm[h][:, :],
                    G[:, 2 * cp:2 * cp + 2, :],
                    at_pair(cp, h),
                    start=(cp == 0),
                    stop=(cp == NJ // 2 - 1),
                    perf_mode=mybir.MatmulPerfMode.DoubleRow,
                )

        def dn_group(cp):
            for h in range(2):
                nc.tensor.matmul(
                    dn[h][:, :],
                    Gg[:, 2 * cp:2 * cp + 2, :],
                    at_pair(cp, h),
                    start=(cp == 0),
                    stop=(cp == NJ // 2 - 1),
                    perf_mode=mybir.MatmulPerfMode.DoubleRow,
                )

        scalar_at = {b: s for s, b in A_ON_SCALAR.items()}
        for b in range(NB):
            ph1_batch(b)
            if b in scalar_at:
                at_dma(nc.scalar, scalar_at[b])
            if b >= NM_LAG:
                nm_group(2 * (b - NM_LAG))
                nm_group(2 * (b - NM_LAG) + 1)
            if b >= DN_LAG:
                dn_group(2 * (b - DN_LAG))
                dn_group(2 * (b - DN_LAG) + 1)
        # drain: dn first so the recip->broadcast->mul chain starts earlier
        for cp in range(2 * (NB - DN_LAG), NJ // 2):
            dn_group(cp)
        for cp in range(2 * (NB - NM_LAG), NJ // 2):
            nm_group(cp)

        # ---- output: out = relu(nm) * (1/dn) broadcast over o ----
        rels, rbcs = [], []
        for h in range(2):
            nc.vector.reciprocal_approx_fast(
                out=rc[0:1, h * HF:(h + 1) * HF], in_=dn[h][0:1, :]
            )
            rel = outp.tile([P, HF], F32, tag="rel", name="rel")
            nc.scalar.activation(rel, nm[h], AFT.Relu)
            rcb = outp.tile([1, HF], BF16, tag="rcb", name="rcb")
            nc.vector.tensor_copy(out=rcb, in_=rc[0:1, h * HF:(h + 1) * HF])
            rbc = ps.tile([P, HF], F32, tag="rbc", name="rbc", bufs=1)
            nc.tensor.matmul(
                rbc, ones_bf[0:1, 0:P], rcb[0:1, :], start=True, stop=True,
            )
            rels.append(rel)
            rbcs.append(rbc)
        for h in range(2):
            o_sb = outp.tile([P, HF], BF16, tag="osb", name="osb")
            nc.vector.tensor_mul(o_sb, rels[h], rbcs[h])
            eng = nc.sync if h == 0 else nc.scalar
            eng.dma_start(out=out[:, h * HF:(h + 1) * HF], in_=o_sb)


def build_nc(repeat=1):
    nc = bacc.Bacc("TRN2", target_bir_lowering=False)
    # at[p, c*RPC + i] = A[core*RPC + i, c*128 + p]  (partition-major repack)
    at = nc.dram_tensor("at", [P, NJ * RPC], FP8, kind="ExternalInput")
    # xt[p, b*1024 + dh*512 + n'] = X[b*512 + n', dh*128 + p]  (batch-major)
    xt = nc.dram_tensor("xt", [P, DH * N], FP8, kind="ExternalInput")
    # wt[p, dh*128 + o] = W.T[dh*128 + p, o]
    wt = nc.dram_tensor("wt", [P, DH * OUT_DIM], BF16, kind="ExternalInput")
    awb = nc.dram_tensor("awb", [P, OUT_DIM], BF16, kind="ExternalInput")
    out = nc.dram_tensor("out", [OUT_DIM, RPC], BF16, kind="ExternalOutput")  # transposed

    with tile.TileContext(nc) as tc:
        with (
            tc.tile_pool(name="big", bufs=1) as big,
            tc.tile_pool(name="ph1", bufs=4) as ph1,
            tc.tile_pool(name="outp", bufs=2) as outp,
        ):
            for _ in range(repeat):
                emit_body(nc, tc, (at, xt, wt, awb, out), (big, ph1, outp))
    nc.compile()
    return nc


_NC_CACHE = None


def _get_nc():
    global _NC_CACHE
    if _NC_CACHE is None:
        _NC_CACHE = build_nc()
    return _NC_CACHE


def make_in_maps(X, A, W, a_w):
    X = np.ascontiguousarray(np.asarray(X, dtype=np.float32))
    A = np.ascontiguousarray(np.asarray(A, dtype=np.float32))
    W = np.ascontiguousarray(np.asarray(W, dtype=np.float32))
    a_w = np.ascontiguousarray(np.asarray(a_w, dtype=np.float32))

    bf = mybir.dt.np(BF16)
    f8 = mybir.dt.np(FP8)
    xtp = (X.T.astype(f8)                        # [256, 8192]
           .reshape(DH, P, NB, B1 * P)           # [dh, p, b, n']
           .transpose(1, 2, 0, 3)                # [p, b, dh, n']
           .reshape(P, DH * N))
    xtp = np.ascontiguousarray(xtp)
    wtp = np.ascontiguousarray(
        W.T.astype(bf)                           # [256, 128]
        .reshape(DH, P, OUT_DIM)                 # [dh, p, o]
        .transpose(1, 0, 2)                      # [p, dh, o]
        .reshape(P, DH * OUT_DIM))
    awp = np.ascontiguousarray(
        np.broadcast_to(a_w[None, :], (P, OUT_DIM)).astype(bf))

    A8 = A.astype(f8)
    in_maps = []
    for c in range(NCORES):
        blk = A8[c * RPC:(c + 1) * RPC, :]       # [i=1024, j=8192]
        atp = (blk.reshape(RPC, NJ, P)           # [i, c, p]
               .transpose(2, 1, 0)               # [p, c, i]
               .reshape(P, NJ * RPC))
        in_maps.append({
            "at": np.ascontiguousarray(atp),
            "xt": xtp, "wt": wtp, "awb": awp,
        })
    return in_maps


def kernel_with_results(X, A, W, a_w, trace=False):
    in_maps = make_in_maps(X, A, W, a_w)
    res = run_bass_kernel_spmd(_get_nc(), in_maps, list(range(NCORES)), trace=trace)
    out = np.concatenate(
        [np.ascontiguousarray(r["out"].T) for r in res.results], axis=0
    )
    return out.astype(np.float32), res


def kernel(X, A, W, a_w):
    out, _ = kernel_with_results(X, A, W, a_w)
    return out


# revision 21
# speedup vs baseline: 1.3382x; 1.0459x over previous
"""GATv2 layer (broadcast-score variant) as a Bass/Tile kernel on 8 NeuronCores.

Math: since scores[i,j] = e[j] (row-broadcast) masked by A, the masked softmax +
aggregation collapse to
    g = exp(e - ln2),  e = relu(X @ W.T) @ a_w          (the ln2 bias cancels)
    out = relu( (A @ (g*Wh)) / (A @ g) )                with Wh = X @ W.T
Each core computes a 1024-row block of the output.

v9 (vs ~64.7us v2 baseline):
- A / X / W repacked on the HOST into [128, *] partition-major layouts so every
  dma_start reads per-partition-contiguous bytes (128 big descriptors/trigger;
  v2's 512-descriptor triggers took ~1us each and serialized the Sync engine,
  starving the DMA queue at 67% duty).
- Whole 8MB A.T block resident in SBUF (no pool recycling); at rides the sync
  HWDGE ring, xt/aw the scalar ring.  Trigger count is kept LOW: Tile recycles
  ~10 DMA semaphores, so extra triggers serialize on prior completions
  (~2us receipt each) — fine slicing measured slower (v6-v8).
- Phase-1 and phase-2 emission interleaved (iter b = ph1 batch b, nm groups of
  batch b-1, dn groups of batch b-2) so the PE queue never drains while the
  DVE/ACT e->g->G chain runs.
- Both nm and dn use fp8 DoubleRow (measured 216ns per 256x512 pass, at peak).
  A col-tiled plain-fp8 dn ran concurrently but MAC-rate-bound -> no win; the
  127 zero Gg columns are the price of DR's full-width stationary.
- Real-contraction warm-up matmuls right after the preamble flip the PE HAM
  clock gate (cold 1.2GHz -> warm 2.4GHz after ~3.4us of sustained activity)
  before real data lands.  (1-contraction dummies do NOT register as activity.)
"""

import numpy as np

import concourse.tile as tile
from concourse import bacc, mybir
from concourse.bass_utils import run_bass_kernel_spmd

N, IN_DIM, OUT_DIM = 8192, 256, 128
NCORES = 8
RPC = N // NCORES          # rows per core (1024)
P = 128                    # partitions
NJ = N // P                # 64 contraction chunks
DH = IN_DIM // P           # 2 chunks of the d-contraction
HF = RPC // 2              # 512-wide i-halves for phase-2 streams
LN2 = 0.6931471805599453

B1 = 4                     # phase-1 j-tile batch (chunks per batch)
NB = NJ // B1              # 16 batches
# at slices in j-chunks: two small leading slices so phase 2 starts early
A_SLICES = [4, 4, 8, 8, 8, 8, 8, 8, 8]
NSL_X = 4                  # xt slices (2048 nodes each)
XCH = NJ // NSL_X          # j-chunks per xt slice (16)

F32 = mybir.dt.float32
BF16 = mybir.dt.bfloat16
FP8 = mybir.dt.float8e4
AFT = mybir.ActivationFunctionType

NM_LAG = 1                 # nm group batch-lag behind phase 1
DN_LAG = 2                 # dn group batch-lag (Gg memset slack)
NWARM = 26                 # warm-up matmuls (128-contraction, ~2.8us cold)


def emit_body(nc, tc, io, pools):
    at, xt, wt, awb, out = io
    big, ph1, outp = pools

    # Gg memset first on the (slow-to-launch) GpSimd queue; split so the
    # leading chunks are zeroed well before the first dn LDWEIGHTS.
    Gg = big.tile([P, NJ, OUT_DIM], FP8, tag="Gg", name="Gg")
    nc.gpsimd.memset(Gg[:, 0:4, :], 0.0)
    nc.gpsimd.memset(Gg[:, 4:16, :], 0.0)
    nc.gpsimd.memset(Gg[:, 16:NJ, :], 0.0)

    # warm-up stationary/moving tile (DVE memset runs right after preamble)
    wu = big.tile([P, P], BF16, tag="wu", name="wu")
    nc.vector.memset(wu, 0.0)

    # wt + first xt piece ride the SYNC ring FIRST (its preamble retires
    # earliest); the first matmul needs exactly wt + xt chunks 0-3.
    wt_sb = big.tile([P, DH, OUT_DIM], BF16, tag="wt_sb", name="wt_sb")
    nc.sync.dma_start(out=wt_sb, in_=wt.rearrange("p (dh o) -> p dh o", dh=DH))
    XSN = N // NSL_X
    xt_s = [big.tile([P, DH, XSN], FP8, tag=f"xt{s}", name=f"xt{s}")
            for s in range(NSL_X)]
    xt0_r = xt[:, 0:DH * XSN].rearrange("p (dh n) -> p dh n", dh=DH)
    nc.sync.dma_start(out=xt_s[0][:, :, 0:512], in_=xt0_r[:, :, 0:512])
    nc.sync.dma_start(out=xt_s[0][:, :, 512:XSN], in_=xt0_r[:, :, 512:XSN])
    # whole A.T column-block -> SBUF on the sync ring behind the xt head
    at_s = []
    at_chunk0 = []
    pos = 0
    for s, ach in enumerate(A_SLICES):
        a4 = big.tile([P, ach, RPC], FP8, tag=f"at{s}", name=f"at{s}")
        nc.sync.dma_start(
            out=a4, in_=at[:, pos * RPC:(pos + ach) * RPC]
            .rearrange("p (c i) -> p c i", c=ach))
        at_s.append(a4)
        at_chunk0.append(pos)
        pos += ach

    def at_pair(cp, h):
        """[P, 2, HF] slice holding j-chunks (2cp, 2cp+1), i-half h."""
        c0 = 2 * cp
        s = 0
        while at_chunk0[s] + A_SLICES[s] <= c0:
            s += 1
        r = c0 - at_chunk0[s]
        return at_s[s][:, r:r + 2, h * HF:(h + 1) * HF]

    # the rest of xt + aw on the scalar HWDGE ring
    aw_sb = big.tile([P, OUT_DIM], BF16, tag="aw_sb", name="aw_sb")
    nc.scalar.dma_start(out=aw_sb, in_=awb[:, :])
    for s in range(1, NSL_X):
        nc.scalar.dma_start(
            out=xt_s[s], in_=xt[:, s * DH * XSN:(s + 1) * DH * XSN]
            .rearrange("p (dh n) -> p dh n", dh=DH))

    # ---- persistent SBUF state --------------------------------------------
    G = big.tile([P, NJ, OUT_DIM], FP8, tag="G", name="G")
    g64 = big.tile([P, NJ], F32, tag="g64", name="g64")
    ones_bf = big.tile([1, P], BF16, tag="ones", name="ones")
    nc.vector.memset(ones_bf, 1.0)
    nln2 = big.tile([P, 1], F32, tag="nln2", name="nln2")
    nc.vector.memset(nln2, -LN2)
    rc = big.tile([1, RPC], F32, tag="rc", name="rc")

    with tc.tile_pool(name="ps", bufs=1, space="PSUM") as ps:
        nm = [ps.tile([P, HF], F32, tag=f"nm{h}", name=f"nm{h}", bufs=1)
              for h in range(2)]
        dn = [ps.tile([P, HF], F32, tag=f"dn{h}", name=f"dn{h}", bufs=1)
              for h in range(2)]

        # HAM warm-up: real 128-contraction matmuls into the (end-of-kernel)
        # rbc bank keep the PE activity monitor busy from preamble-end until
        # real data lands, so the clock gate opens to 2.4GHz early.
        warm = ps.tile([P, HF], F32, tag="rbc", name="warm", bufs=1)
        for _ in range(NWARM):
            nc.tensor.matmul(
                warm[:, 0:P], wu, wu, start=True, stop=True,
            )

        def ph1_batch(b):
            wh4 = ps.tile([P, B1, OUT_DIM], F32, tag="wh4", name="wh4", bufs=3)
            for k in range(B1):
                t = b * B1 + k
                s, off = t // XCH, (t % XCH) * P
                for dh in range(DH):
                    nc.tensor.matmul(
                        wh4[:, k, :],
                        xt_s[s][:, dh, off:off + P],
                        wt_sb[:, dh, :],
                        start=(dh == 0),
                        stop=(dh == DH - 1),
                    )
            t0 = b * B1
            scr = ph1.tile([P, B1, OUT_DIM], FP8, name="scr")
            e4 = ph1.tile([P, B1], F32, name="e4")
            for k in range(B1):
                nc.vector.scalar_tensor_tensor(
                    out=scr[:, k, :], in0=wh4[:, k, :], scalar=0.0,
                    in1=aw_sb,
                    op0=mybir.AluOpType.max, op1=mybir.AluOpType.mult,
                    accum_out=e4[:, k:k + 1],
                )
            nc.scalar.activation(g64[:, t0:t0 + B1], e4, AFT.Exp, bias=nln2[:, 0:1])
            for k in range(B1):
                t = t0 + k
                # 3-of-4 G-copies on Scalar, 1-of-4 on DVE (DVE owns the e-STTs)
                if t % 4 == 3:
                    nc.vector.tensor_scalar_mul(
                        G[:, t, :], wh4[:, k, :], g64[:, t:t + 1]
                    )
                else:
                    nc.scalar.activation(
                        G[:, t, :], wh4[:, k, :], AFT.Copy,
                        scale=g64[:, t:t + 1],
                    )
            nc.gpsimd.tensor_copy(
                out=Gg[:, t0:t0 + B1, 0:1], in_=g64[:, t0:t0 + B1]
            )

        def nm_group(cp):
            for h in range(2):
                nc.tensor.matmul(
                    nm[h][:, :],
                    G[:, 2 * cp:2 * cp + 2, :],
                    at_pair(cp, h),
                    start=(cp == 0),
                    stop=(cp == NJ // 2 - 1),
                    perf_mode=mybir.MatmulPerfMode.DoubleRow,
                )

        def dn_group(cp):
            for h in range(2):
                nc.tensor.matmul(
                    dn[h][:, :],
                    Gg[:, 2 * cp:2 * cp + 2, :],
                    at_pair(cp, h),
                    start=(cp == 0),
                    stop=(cp == NJ // 2 - 1),
                    perf_mode=mybir.MatmulPerfMode.DoubleRow,
                )

        for b in range(NB):
            ph1_batch(b)
            if b >= NM_LAG:
                nm_group(2 * (b - NM_LAG))
                nm_group(2 * (b - NM_LAG) + 1)
            if b >= DN_LAG:
                dn_group(2 * (b - DN_LAG))
                dn_group(2 * (b - DN_LAG) + 1)
        # drain: dn first so the recip->broadcast->mul chain starts earlier
        for cp in range(2 * (NB - DN_LAG), NJ // 2):
            dn_group(cp)
        for cp in range(2 * (NB - NM_LAG), NJ // 2):
            nm_group(cp)

        # ---- output: out = relu(nm) * (1/dn) broadcast over o ----
        rels, rbcs = [], []
        for h in range(2):
            nc.vector.reciprocal_approx_fast(
                out=rc[0:1, h * HF:(h + 1) * HF], in_=dn[h][0:1, :]
            )
            rel = outp.tile([P, HF], F32, tag="rel", name="rel")
            nc.scalar.activation(rel, nm[h], AFT.Relu)
            rcb = outp.tile([1, HF], BF16, tag="rcb", name="rcb")
            nc.vector.tensor_copy(out=rcb, in_=rc[0:1, h * HF:(h + 1) * HF])
            rbc = ps.tile([P, HF], F32, tag="rbc", name="rbc", bufs=1)
            nc.tensor.matmul(
                rbc, ones_bf[0:1, 0:P], rcb[0:1, :], start=True, stop=True,
            )
            rels.append(rel)
            rbcs.append(rbc)
        for h in range(2):
            o_sb = outp.tile([P, HF], BF16, tag="osb", name="osb")
            nc.vector.tensor_mul(o_sb, rels[h], rbcs[h])
            eng = nc.sync if h == 0 else nc.scalar
            eng.dma_start(out=out[:, h * HF:(h + 1) * HF], in_=o_sb)


def build_nc(repeat=1):
    nc = bacc.Bacc("TRN2", target_bir_lowering=False)
    # at[p, c*RPC + i] = A[core*RPC + i, c*128 + p]  (partition-major repack)
    at = nc.dram_tensor("at", [P, NJ * RPC], FP8, kind="ExternalInput")
    # xt[p, s*4096 + dh*2048 + n'] = X[s*2048 + n', dh*128 + p]  (slice-major)
    xt = nc.dram_tensor("xt", [P, DH * N], FP8, kind="ExternalInput")
    # wt[p, dh*128 + o] = W.T[dh*128 + p, o]
    wt = nc.dram_tensor("wt", [P, DH * OUT_DIM], BF16, kind="ExternalInput")
    awb = nc.dram_tensor("awb", [P, OUT_DIM], BF16, kind="ExternalInput")
    out = nc.dram_tensor("out", [OUT_DIM, RPC], BF16, kind="ExternalOutput")  # transposed

    with tile.TileContext(nc) as tc:
        with (
            tc.tile_pool(name="big", bufs=1) as big,
            tc.tile_pool(name="ph1", bufs=4) as ph1,
            tc.tile_pool(name="outp", bufs=2) as outp,
        ):
            for _ in range(repeat):
                emit_body(nc, tc, (at, xt, wt, awb, out), (big, ph1, outp))
    nc.compile()
    return nc


_NC_CACHE = None


def _get_nc():
    global _NC_CACHE
    if _NC_CACHE is None:
        _NC_CACHE = build_nc()
    return _NC_CACHE


def make_in_maps(X, A, W, a_w):
    X = np.ascontiguousarray(np.asarray(X, dtype=np.float32))
    A = np.ascontiguousarray(np.asarray(A, dtype=np.float32))
    W = np.ascontiguousarray(np.asarray(W, dtype=np.float32))
    a_w = np.ascontiguousarray(np.asarray(a_w, dtype=np.float32))

    bf = mybir.dt.np(BF16)
    f8 = mybir.dt.np(FP8)
    NSX = N // NSL_X
    xtp = (X.T.astype(f8)                        # [256, 8192]
           .reshape(DH, P, NSL_X, NSX)           # [dh, p, s, n']
           .transpose(1, 2, 0, 3)                # [p, s, dh, n']
           .reshape(P, DH * N))
    xtp = np.ascontiguousarray(xtp)
    wtp = np.ascontiguousarray(
        W.T.astype(bf)                           # [256, 128]
        .reshape(DH, P, OUT_DIM)                 # [dh, p, o]
        .transpose(1, 0, 2)                      # [p, dh, o]
        .reshape(P, DH * OUT_DIM))
    awp = np.ascontiguousarray(
        np.broadcast_to(a_w[None, :], (P, OUT_DIM)).astype(bf))

    A8 = A.astype(f8)
    in_maps = []
    for c in range(NCORES):
        blk = A8[c * RPC:(c + 1) * RPC, :]       # [i=1024, j=8192]
        atp = (blk.reshape(RPC, NJ, P)           # [i, c, p]
               .transpose(2, 1, 0)               # [p, c, i]
               .reshape(P, NJ * RPC))
        in_maps.append({
            "at": np.ascontiguousarray(atp),
            "xt": xtp, "wt": wtp, "awb": awp,
        })
    return in_maps


def kernel_with_results(X, A, W, a_w, trace=False):
    in_maps = make_in_maps(X, A, W, a_w)
    res = run_bass_kernel_spmd(_get_nc(), in_maps, list(range(NCORES)), trace=trace)
    out = np.concatenate(
        [np.ascontiguousarray(r["out"].T) for r in res.results], axis=0
    )
    return out.astype(np.float32), res


def kernel(X, A, W, a_w):
    out, _ = kernel_with_results(X, A, W, a_w)
    return out


# revision 25
# speedup vs baseline: 1.3470x; 1.0065x over previous
"""GATv2 layer (broadcast-score variant) as a Bass/Tile kernel on 8 NeuronCores.

Math: since scores[i,j] = e[j] (row-broadcast) masked by A, the masked softmax +
aggregation collapse to
    g = exp(e - ln2),  e = relu(X @ W.T) @ a_w          (the ln2 bias cancels)
    out = relu( (A @ (g*Wh)) / (A @ g) )                with Wh = X @ W.T
Each core computes a 1024-row block of the output.

v9 (vs ~64.7us v2 baseline):
- A / X / W repacked on the HOST into [128, *] partition-major layouts so every
  dma_start reads per-partition-contiguous bytes (128 big descriptors/trigger;
  v2's 512-descriptor triggers took ~1us each and serialized the Sync engine,
  starving the DMA queue at 67% duty).
- Whole 8MB A.T block resident in SBUF (no pool recycling); at rides the sync
  HWDGE ring, xt/aw the scalar ring.  Trigger count is kept LOW: Tile recycles
  ~10 DMA semaphores, so extra triggers serialize on prior completions
  (~2us receipt each) — fine slicing measured slower (v6-v8).
- Phase-1 and phase-2 emission interleaved (iter b = ph1 batch b, nm groups of
  batch b-1, dn groups of batch b-2) so the PE queue never drains while the
  DVE/ACT e->g->G chain runs.
- Both nm and dn use fp8 DoubleRow (measured 216ns per 256x512 pass, at peak).
  A col-tiled plain-fp8 dn ran concurrently but MAC-rate-bound -> no win; the
  127 zero Gg columns are the price of DR's full-width stationary.
- Real-contraction warm-up matmuls right after the preamble flip the PE HAM
  clock gate (cold 1.2GHz -> warm 2.4GHz after ~3.4us of sustained activity)
  before real data lands.  (1-contraction dummies do NOT register as activity.)
"""

import numpy as np

import concourse.tile as tile
from concourse import bacc, mybir
from concourse.bass_utils import run_bass_kernel_spmd

N, IN_DIM, OUT_DIM = 8192, 256, 128
NCORES = 8
RPC = N // NCORES          # rows per core (1024)
P = 128                    # partitions
NJ = N // P                # 64 contraction chunks
DH = IN_DIM // P           # 2 chunks of the d-contraction
HF = RPC // 2              # 512-wide i-halves for phase-2 streams
LN2 = 0.6931471805599453

B1 = 4                     # phase-1 j-tile batch (chunks per batch)
NB = NJ // B1              # 16 batches
# at slices in j-chunks: two small leading slices so phase 2 starts early
A_SLICES = [4, 4, 8, 8, 8, 8, 8, 8, 8]
NSL_X = 4                  # xt slices (2048 nodes each)
XCH = NJ // NSL_X          # j-chunks per xt slice (16)

F32 = mybir.dt.float32
BF16 = mybir.dt.bfloat16
FP8 = mybir.dt.float8e4
AFT = mybir.ActivationFunctionType

NM_LAG = 1                 # nm group batch-lag behind phase 1
DN_LAG = 2                 # dn group batch-lag (Gg memset slack)
# warm-up matmuls (real 128-contraction, N=512, ~427ns each cold).  The
# initial burst runs preamble-end -> first data; the per-batch fills keep the
# PE "busy" through the early DMA-receipt waits so the HAM activity window
# stays hot and the clock gate opens at ~2.4GHz by batch 3 (measured flip at
# t=22us without fills -> half-clock until then).
WARM0 = 7
WARM_FILL = {0: 4, 1: 4, 2: 3}


def emit_body(nc, tc, io, pools):
    at, xt, wt, awb, out = io
    big, ph1, outp = pools

    # Gg memset first on the (slow-to-launch) GpSimd queue; split so the
    # leading chunks are zeroed well before the first dn LDWEIGHTS.
    Gg = big.tile([P, NJ, OUT_DIM], FP8, tag="Gg", name="Gg")
    nc.gpsimd.memset(Gg[:, 0:4, :], 0.0)
    nc.gpsimd.memset(Gg[:, 4:16, :], 0.0)
    nc.gpsimd.memset(Gg[:, 16:NJ, :], 0.0)

    # warm-up stationary/moving tile (DVE memset runs right after preamble)
    wu = big.tile([P, HF], BF16, tag="wu", name="wu")
    nc.vector.memset(wu, 0.0)

    # wt + first xt piece ride the SYNC ring FIRST (its preamble retires
    # earliest); the first matmul needs exactly wt + xt chunks 0-3.
    wt_sb = big.tile([P, DH, OUT_DIM], BF16, tag="wt_sb", name="wt_sb")
    nc.sync.dma_start(out=wt_sb, in_=wt.rearrange("p (dh o) -> p dh o", dh=DH))
    XSN = N // NSL_X
    xt_s = [big.tile([P, DH, XSN], FP8, tag=f"xt{s}", name=f"xt{s}")
            for s in range(NSL_X)]
    xt0_r = xt[:, 0:DH * XSN].rearrange("p (dh n) -> p dh n", dh=DH)
    nc.sync.dma_start(out=xt_s[0][:, :, 0:512], in_=xt0_r[:, :, 0:512])
    nc.sync.dma_start(out=xt_s[0][:, :, 512:XSN], in_=xt0_r[:, :, 512:XSN])
    # whole A.T column-block -> SBUF on the sync ring behind the xt head
    at_s = []
    at_chunk0 = []
    pos = 0
    for s, ach in enumerate(A_SLICES):
        a4 = big.tile([P, ach, RPC], FP8, tag=f"at{s}", name=f"at{s}")
        nc.sync.dma_start(
            out=a4, in_=at[:, pos * RPC:(pos + ach) * RPC]
            .rearrange("p (c i) -> p c i", c=ach))
        at_s.append(a4)
        at_chunk0.append(pos)
        pos += ach

    def at_pair(cp, h):
        """[P, 2, HF] slice holding j-chunks (2cp, 2cp+1), i-half h."""
        c0 = 2 * cp
        s = 0
        while at_chunk0[s] + A_SLICES[s] <= c0:
            s += 1
        r = c0 - at_chunk0[s]
        return at_s[s][:, r:r + 2, h * HF:(h + 1) * HF]

    # the rest of xt + aw on the scalar HWDGE ring
    aw_sb = big.tile([P, OUT_DIM], BF16, tag="aw_sb", name="aw_sb")
    nc.scalar.dma_start(out=aw_sb, in_=awb[:, :])
    for s in range(1, NSL_X):
        nc.scalar.dma_start(
            out=xt_s[s], in_=xt[:, s * DH * XSN:(s + 1) * DH * XSN]
            .rearrange("p (dh n) -> p dh n", dh=DH))

    # ---- persistent SBUF state --------------------------------------------
    G = big.tile([P, NJ, OUT_DIM], FP8, tag="G", name="G")
    g64 = big.tile([P, NJ], F32, tag="g64", name="g64")
    ones_bf = big.tile([1, P], BF16, tag="ones", name="ones")
    nc.vector.memset(ones_bf, 1.0)
    nln2 = big.tile([P, 1], F32, tag="nln2", name="nln2")
    nc.vector.memset(nln2, -LN2)
    rc = big.tile([1, RPC], F32, tag="rc", name="rc")

    with tc.tile_pool(name="ps", bufs=1, space="PSUM") as ps:
        nm = [ps.tile([P, HF], F32, tag=f"nm{h}", name=f"nm{h}", bufs=1)
              for h in range(2)]
        dn = [ps.tile([P, HF], F32, tag=f"dn{h}", name=f"dn{h}", bufs=1)
              for h in range(2)]

        # HAM warm-up: real 128-contraction matmuls into the (end-of-kernel)
        # rbc bank keep the PE activity monitor busy from preamble-end until
        # real data lands, so the clock gate opens to 2.4GHz early.
        warm = ps.tile([P, HF], F32, tag="rbc", name="warm", bufs=1)

        def warm_burst(n):
            for _ in range(n):
                nc.tensor.matmul(
                    warm[:, :], wu[:, 0:P], wu, start=True, stop=True,
                )

        warm_burst(WARM0)

        def ph1_batch(b):
            wh4 = ps.tile([P, B1, OUT_DIM], F32, tag="wh4", name="wh4", bufs=3)
            for k in range(B1):
                t = b * B1 + k
                s, off = t // XCH, (t % XCH) * P
                for dh in range(DH):
                    nc.tensor.matmul(
                        wh4[:, k, :],
                        xt_s[s][:, dh, off:off + P],
                        wt_sb[:, dh, :],
                        start=(dh == 0),
                        stop=(dh == DH - 1),
                    )
            t0 = b * B1
            scr = ph1.tile([P, B1, OUT_DIM], FP8, name="scr")
            e4 = ph1.tile([P, B1], F32, name="e4")
            for k in range(B1):
                nc.vector.scalar_tensor_tensor(
                    out=scr[:, k, :], in0=wh4[:, k, :], scalar=0.0,
                    in1=aw_sb,
                    op0=mybir.AluOpType.max, op1=mybir.AluOpType.mult,
                    accum_out=e4[:, k:k + 1],
                )
            nc.scalar.activation(g64[:, t0:t0 + B1], e4, AFT.Exp, bias=nln2[:, 0:1])
            for k in range(B1):
                t = t0 + k
                # 3-of-4 G-copies on Scalar, 1-of-4 on DVE (DVE owns the e-STTs)
                if t % 4 == 3:
                    nc.vector.tensor_scalar_mul(
                        G[:, t, :], wh4[:, k, :], g64[:, t:t + 1]
                    )
                else:
                    nc.scalar.activation(
                        G[:, t, :], wh4[:, k, :], AFT.Copy,
                        scale=g64[:, t:t + 1],
                    )
            nc.gpsimd.tensor_copy(
                out=Gg[:, t0:t0 + B1, 0:1], in_=g64[:, t0:t0 + B1]
            )

        def nm_group(cp):
            for h in range(2):
                nc.tensor.matmul(
                    nm[h][:, :],
                    G[:, 2 * cp:2 * cp + 2, :],
                    at_pair(cp, h),
                    start=(cp == 0),
                    stop=(cp == NJ // 2 - 1),
                    perf_mode=mybir.MatmulPerfMode.DoubleRow,
                )

        def dn_group(cp):
            for h in range(2):
                nc.tensor.matmul(
                    dn[h][:, :],
                    Gg[:, 2 * cp:2 * cp + 2, :],
                    at_pair(cp, h),
                    start=(cp == 0),
                    stop=(cp == NJ // 2 - 1),
                    perf_mode=mybir.MatmulPerfMode.DoubleRow,
                )

        for b in range(NB):
            ph1_batch(b)
            if b in WARM_FILL:
                warm_burst(WARM_FILL[b])
            if b >= NM_LAG:
                nm_group(2 * (b - NM_LAG))
                nm_group(2 * (b - NM_LAG) + 1)
            if b >= DN_LAG:
                dn_group(2 * (b - DN_LAG))
                dn_group(2 * (b - DN_LAG) + 1)
        # drain: dn first so the recip->broadcast->mul chain starts earlier
        for cp in range(2 * (NB - DN_LAG), NJ // 2):
            dn_group(cp)
        for cp in range(2 * (NB - NM_LAG), NJ // 2):
            nm_group(cp)

        # ---- output: out = relu(nm) * (1/dn) broadcast over o ----
        rels, rbcs = [], []
        for h in range(2):
            nc.vector.reciprocal_approx_fast(
                out=rc[0:1, h * HF:(h + 1) * HF], in_=dn[h][0:1, :]
            )
            rel = outp.tile([P, HF], F32, tag="rel", name="rel")
            nc.scalar.activation(rel, nm[h], AFT.Relu)
            rcb = outp.tile([1, HF], BF16, tag="rcb", name="rcb")
            nc.vector.tensor_copy(out=rcb, in_=rc[0:1, h * HF:(h + 1) * HF])
            rbc = ps.tile([P, HF], F32, tag="rbc", name="rbc", bufs=1)
            nc.tensor.matmul(
                rbc, ones_bf[0:1, 0:P], rcb[0:1, :], start=True, stop=True,
            )
            rels.append(rel)
            rbcs.append(rbc)
        for h in range(2):
            o_sb = outp.tile([P, HF], BF16, tag="osb", name="osb")
            nc.vector.tensor_mul(o_sb, rels[h], rbcs[h])
            eng = nc.sync if h == 0 else nc.scalar
            eng.dma_start(out=out[:, h * HF:(h + 1) * HF], in_=o_sb)


def build_nc(repeat=1):
    nc = bacc.Bacc("TRN2", target_bir_lowering=False)
    # at[p, c*RPC + i] = A[core*RPC + i, c*128 + p]  (partition-major repack)
    at = nc.dram_tensor("at", [P, NJ * RPC], FP8, kind="ExternalInput")
    # xt[p, s*4096 + dh*2048 + n'] = X[s*2048 + n', dh*128 + p]  (slice-major)
    xt = nc.dram_tensor("xt", [P, DH * N], FP8, kind="ExternalInput")
    # wt[p, dh*128 + o] = W.T[dh*128 + p, o]
    wt = nc.dram_tensor("wt", [P, DH * OUT_DIM], BF16, kind="ExternalInput")
    awb = nc.dram_tensor("awb", [P, OUT_DIM], BF16, kind="ExternalInput")
    out = nc.dram_tensor("out", [OUT_DIM, RPC], BF16, kind="ExternalOutput")  # transposed

    with tile.TileContext(nc) as tc:
        with (
            tc.tile_pool(name="big", bufs=1) as big,
            tc.tile_pool(name="ph1", bufs=4) as ph1,
            tc.tile_pool(name="outp", bufs=2) as outp,
        ):
            for _ in range(repeat):
                emit_body(nc, tc, (at, xt, wt, awb, out), (big, ph1, outp))
    nc.compile()
    return nc


_NC_CACHE = None


def _get_nc():
    global _NC_CACHE
    if _NC_CACHE is None:
        _NC_CACHE = build_nc()
    return _NC_CACHE


def make_in_maps(X, A, W, a_w):
    X = np.ascontiguousarray(np.asarray(X, dtype=np.float32))
    A = np.ascontiguousarray(np.asarray(A, dtype=np.float32))
    W = np.ascontiguousarray(np.asarray(W, dtype=np.float32))
    a_w = np.ascontiguousarray(np.asarray(a_w, dtype=np.float32))

    bf = mybir.dt.np(BF16)
    f8 = mybir.dt.np(FP8)
    NSX = N // NSL_X
    xtp = (X.T.astype(f8)                        # [256, 8192]
           .reshape(DH, P, NSL_X, NSX)           # [dh, p, s, n']
           .transpose(1, 2, 0, 3)                # [p, s, dh, n']
           .reshape(P, DH * N))
    xtp = np.ascontiguousarray(xtp)
    wtp = np.ascontiguousarray(
        W.T.astype(bf)                           # [256, 128]
        .reshape(DH, P, OUT_DIM)                 # [dh, p, o]
        .transpose(1, 0, 2)                      # [p, dh, o]
        .reshape(P, DH * OUT_DIM))
    awp = np.ascontiguousarray(
        np.broadcast_to(a_w[None, :], (P, OUT_DIM)).astype(bf))

    A8 = A.astype(f8)
    in_maps = []
    for c in range(NCORES):
        blk = A8[c * RPC:(c + 1) * RPC, :]       # [i=1024, j=8192]
        atp = (blk.reshape(RPC, NJ, P)           # [i, c, p]
               .transpose(2, 1, 0)               # [p, c, i]
               .reshape(P, NJ * RPC))
        in_maps.append({
            "at": np.ascontiguousarray(atp),
            "xt": xtp, "wt": wtp, "awb": awp,
        })
    return in_maps


def kernel_with_results(X, A, W, a_w, trace=False):
    in_maps = make_in_maps(X, A, W, a_w)
    res = run_bass_kernel_spmd(_get_nc(), in_maps, list(range(NCORES)), trace=trace)
    out = np.concatenate(
        [np.ascontiguousarray(r["out"].T) for r in res.results], axis=0
    )
    return out.astype(np.float32), res


def kernel(X, A, W, a_w):
    out, _ = kernel_with_results(X, A, W, a_w)
    return out
